# revision 1
# baseline (speedup 1.0000x reference)
"""ABMIL attention pooling on 8 TRN2 NeuronCores (Bass/Tile, SPMD).

Reference (per bag b over N=16384 instances):
    a_n   = tanh(x_n . w1) * sigmoid(x_n . w2)     gated attention score
    att   = softmax over valid n of a              (invalid -> -1e4)
    out_b = sum_n att_n * (x_n @ wf.T)             weighted pooling + proj

Algebraic folds that make this memory-bound (read xs exactly once):
  * out = (sum_n att_n x_n) @ wf.T == sum_n att_n (x_n @ wf.T), so the only
    large compute is ONE matmul y = xs @ [wf.T | 2*w1 | w2] ([N, 6]); pooling
    then reduces tiny [N, 6] data.
  * scores lie in (-1, 1), so softmax needs no max-subtraction:
    out = sum(e*y) / sum_valid(e) with e = exp(a).
  * sigmoid(x) = 0.5*(1 + tanh(x/2)) keeps all transcendentals in one ACT
    table set (tanh + exp); w1 is pre-scaled by 2 on the host so a uniform
    0.5 activation scale serves both tanh rows.
  * invalid instances are zeroed in the packed xs, so their y rows are 0 and
    e = exp(0) = 1 exactly; the host epilogue subtracts the known per-bag
    invalid count from the denominator. No mask tensor reaches the device.

Sharding (flash-attention style, per the problem hint): the instance dim N is
split 8 ways; each core streams its 32 MiB xs shard once and produces 20
partial floats (per bag: sum e and sum e*y). kernel() sums the 8 partial
stat vectors and finalizes out = t/s on the host (the "unshard" step).

Per-core pipeline (~95 us DMA roofline at the measured ~400 GB/s):
  * host pre-packs the shard transposed/tiled so every DMA is a contiguous
    [128, 4096] load; the f32->bf16 cast rides the SWDGE DMA at line rate.
  * per 512-instance tile: one 6-wide PE accumulation group (8 chunk matmuls
    over D=1024, bf16) -> psY[6, 512] in PSUM. Tiles are processed in
    groups of 4 so PE bursts are long enough to hold the HAM clock warm.
  * engine ops need partition bases == 0 mod 32, so the two score rows are
    tanh-ed in place ([6, NT] ACT op) and then each broadcast to a base-0
    tile with a DVE stream_shuffle; e is shuffle-broadcast to 4 partitions
    for the numerator multiply.
  * ACT's accum_out fuses the denominator reduction into the exp; a DVE
    scalar_tensor_tensor with accum_out fuses multiply+reduce for the
    numerator. Per-bag partial columns fold as soon as a bag completes.
"""

import numpy as np

B, N, D, L = 4, 16384, 1024, 4
NCORES = 8
NSH = N // NCORES            # 2048 instances per bag per core
J = B * NSH                  # 8192 flattened rows per core
NT = 512                     # instances per compute tile (1 PSUM bank)
T = J // NT                  # 16 tiles
C8 = D // 128                # 8 contraction chunks of 128
TPB = NSH // NT              # tiles per bag = 4

_NC_CACHE = {}


def _build_nc():
    from concourse import bacc, mybir, tile

    dt = mybir.dt
    act = mybir.ActivationFunctionType
    alu = mybir.AluOpType
    f32 = dt.float32
    bf16 = dt.bfloat16

    nc = bacc.Bacc(
        "TRN2", target_bir_lowering=False, debug=False, num_devices=NCORES
    )

    xsp = nc.dram_tensor("xsp", [T, 128, C8 * NT], f32, kind="ExternalInput").ap()
    # [128, 48]: per chunk c, cols c*6..c*6+5 = [wf0..wf3, w1, w2]
    wsb = nc.dram_tensor("wsb", [128, C8 * 6], bf16, kind="ExternalInput").ap()
    # per-core partial stats: [s_b (4) | t_{b,l} l-major (16)]
    outp = nc.dram_tensor("out", [1, 5 * B], f32, kind="ExternalOutput").ap()

    with tile.TileContext(nc) as tc:
        with (
            tc.tile_pool(name="const", bufs=1) as constp,
            tc.tile_pool(name="xs", bufs=2) as xpool,
            tc.tile_pool(name="psY", bufs=2, space="PSUM") as ypool,
            tc.tile_pool(name="sm", bufs=3) as smp,
        ):
            w_sb = constp.tile([128, C8 * 6], bf16, tag="w")
            nc.sync.dma_start(w_sb[:], wsb)
            sPP = constp.tile([1, T], f32, tag="sPP")
            sYY = constp.tile([4, T], f32, tag="sYY")
            redP = constp.tile([1, B], f32, tag="redP")
            redY = constp.tile([4, B], f32, tag="redY")

            # group sizes: big bursts keep the PE's HAM clock warm during
            # the stream; the taper at the end stops the last tiles' post-
            # processing chains from bunching up after the final matmuls
            GROUPS = [4, 4, 4, 2, 1, 1]
            assert sum(GROUPS) == T
            starts = [sum(GROUPS[:i]) for i in range(len(GROUPS))]
            for gi, g0 in enumerate(starts):
              grp = range(g0, g0 + GROUPS[gi])
              xts, psYs = [], []
              for t in grp:
                xt = xpool.tile([128, C8 * NT], bf16, tag=f"xt{t % 4}")
                nc.gpsimd.dma_start(xt[:], xsp[t])  # f32 -> bf16 cast DMA
                xts.append(xt)
              for t in grp:
                xt = xts[t - g0]
                psY = ypool.tile([6, NT], f32, tag=f"y{t % 4}")
                psYs.append(psY)
                for c in range(C8):
                    nc.tensor.matmul(
                        psY[:],
                        w_sb[:, c * 6 : (c + 1) * 6],
                        xt[:, c * NT : (c + 1) * NT],
                        start=(c == 0),
                        stop=(c == C8 - 1),
                    )
              for t in grp:
                bg = t // TPB  # bag index for this tile
                psY = psYs[t - g0]

                # tanh(y/2) of all 6 rows (only rows 4,5 used; host scaled
                # w1 by 2 so row4 gives tanh(s1) under the uniform /2 scale)
                tts = smp.tile([32, NT], bf16, tag="tts")
                nc.scalar.activation(tts[0:6, :], psY[:], act.Tanh, scale=0.5)
                # shuffle-broadcast rows 4 and 5 each to a base-0 tile
                uA = smp.tile([32, NT], bf16, tag="uA")
                nc.vector.stream_shuffle(uA[:], tts[:], [4] * 32)
                uB = smp.tile([32, NT], bf16, tag="uB")
                nc.vector.stream_shuffle(uB[:], tts[:], [5] * 32)
                # v = tanh(s1) * (tanh(s2/2) + 1) == 2*a ; e = exp(v/2)
                # invalid instances have zeroed xs -> e = exp(0) = 1 exactly;
                # the host subtracts the per-bag invalid count from sum(e)
                v = smp.tile([1, NT], f32, tag="v")
                nc.vector.scalar_tensor_tensor(
                    v[:], uB[0:1, :], 1.0, uA[0:1, :], alu.add, alu.mult
                )
                # e = exp(v/2); denominator partial = sum_n e (accum_out)
                e = smp.tile([32, NT], bf16, tag="e")
                nc.scalar.activation(
                    e[0:1, :], v[:], act.Exp, scale=0.5,
                    accum_out=sPP[0:1, t : t + 1],
                )
                # broadcast e to 4 partitions
                ebs = smp.tile([32, NT], bf16, tag="ebs")
                nc.vector.stream_shuffle(ebs[:], e[:], [0] * 32)

                # numerator partials: sYY[:, t] = sum_n psY[0:4]*e
                jY = smp.tile([4, NT], f32, tag="jY")
                nc.vector.scalar_tensor_tensor(
                    jY[:], psY[0:4, :], 1.0, ebs[0:4, :], alu.mult, alu.mult,
                    accum_out=sYY[0:4, t : t + 1],
                )
                if t % TPB == TPB - 1:
                    # bag bg complete: fold its 4 partial columns immediately
                    nc.vector.tensor_reduce(
                        redP[0:1, bg : bg + 1],
                        sPP[0:1, bg * TPB : (bg + 1) * TPB],
                        mybir.AxisListType.X,
                        alu.add,
                    )
                    nc.vector.tensor_reduce(
                        redY[0:4, bg : bg + 1],
                        sYY[0:4, bg * TPB : (bg + 1) * TPB],
                        mybir.AxisListType.X,
                        alu.add,
                    )
                    # ship this bag's output columns now, off the tail path
                    # (out layout: flat[0:4]=s_b; flat[4+l*4+b]=t_{b,l})
                    nc.scalar.dma_start(
                        outp[0:1, bg : bg + 1], redP[0:1, bg : bg + 1]
                    )
                    nc.sync.dma_start(
                        outp[0:1, B : 5 * B].rearrange("p (l b) -> l (p b)", l=4)[
                            :, bg : bg + 1
                        ],
                        redY[0:4, bg : bg + 1],
                    )



    nc.compile()
    return nc


def _get_nc():
    if "nc" not in _NC_CACHE:
        _NC_CACHE["nc"] = _build_nc()
    return _NC_CACHE["nc"]


def _make_in_maps(xs, valid, w1, w2, wf):
    import ml_dtypes

    validf = valid.astype(np.float32)
    xsz = (xs.astype(np.float32) * validf[..., None]).astype(np.float32)
    # [D, 6] = [wf.T | w1 | w2], packed per 128-chunk: (p, c*6+j) = W6[c*128+p, j]
    W6 = np.concatenate(
        [wf.astype(np.float32).T, 2.0 * w1.astype(np.float32), w2.astype(np.float32)],
        axis=1,
    )
    wsb = np.ascontiguousarray(
        W6.reshape(C8, 128, 6).transpose(1, 0, 2).reshape(128, C8 * 6)
    ).astype(ml_dtypes.bfloat16)


    in_maps = []
    for c in range(NCORES):
        sh = xsz[:, c * NSH : (c + 1) * NSH, :].reshape(J, D)
        xt = sh.T  # [D, J]
        packed = np.ascontiguousarray(
            xt.reshape(C8, 128, T, NT).transpose(2, 1, 0, 3)
        ).reshape(T, 128, C8 * NT)
        in_maps.append({"xsp": packed, "wsb": wsb})
    return in_maps


def _run(xs, valid, w1, w2, wf, trace=False, **kwargs):
    from concourse import bass_utils

    nc = _get_nc()
    in_maps = _make_in_maps(xs, valid, w1, w2, wf)
    res = bass_utils.run_bass_kernel_spmd(
        nc, in_maps, core_ids=list(range(NCORES)), trace=trace, **kwargs
    )
    return res


def _combine(res, valid):
    """Sum per-core partial stats (flash-style unshard) and finalize t/s.

    Invalid instances were zeroed on the device input, so each contributes
    exp(0) = 1 to the denominator partials; subtract their count here.
    """
    tot = np.zeros(5 * B, np.float64)
    for c in range(NCORES):
        tot += np.asarray(res.results[c]["out"]).reshape(5 * B).astype(np.float64)
    n_invalid = (~valid.astype(bool)).sum(axis=1).astype(np.float64)  # [b]
    s = tot[0:B] - n_invalid                      # [b]
    t = tot[B:].reshape(L, B).T                   # [b, l]
    return (t / s[:, None]).astype(np.float32)


def kernel(xs, valid, w1, w2, wf):
    xs, valid, w1, w2, wf = (np.asarray(a) for a in (xs, valid, w1, w2, wf))
    res = _run(xs, valid, w1, w2, wf, trace=False)
    return _combine(res, valid)



# revision 3
# speedup vs baseline: 1.3800x; 1.3800x over previous
"""ABMIL attention pooling on 8 TRN2 NeuronCores (Bass/Tile, SPMD).

Reference (per bag b over N=16384 instances):
    a_n   = tanh(x_n . w1) * sigmoid(x_n . w2)     gated attention score
    att   = softmax over valid n of a              (invalid -> -1e4)
    out_b = sum_n att_n * (x_n @ wf.T)             weighted pooling + proj

Algebraic folds that make this memory-bound (read xs exactly once):
  * out = (sum_n att_n x_n) @ wf.T == sum_n att_n (x_n @ wf.T), so the only
    large compute is ONE matmul y = xs @ [wf.T | 2*w1 | w2] ([N, 6]); pooling
    then reduces tiny [N, 6] data.
  * scores lie in (-1, 1), so softmax needs no max-subtraction:
    out = sum(e*y) / sum_valid(e) with e = exp(a).
  * sigmoid(x) = 0.5*(1 + tanh(x/2)) keeps all transcendentals in one ACT
    table set (tanh + exp); w1 is pre-scaled by 2 on the host so a uniform
    0.5 activation scale serves both tanh rows.
  * invalid instances are zeroed in the packed xs, so their y rows are 0 and
    e = exp(0) = 1 exactly; the host epilogue subtracts the known per-bag
    invalid count from the denominator. No mask tensor reaches the device.

Sharding (flash-attention style, per the problem hint): the instance dim N is
split 8 ways; each core streams its 32 MiB xs shard once and produces 20
partial floats (per bag: sum e and sum e*y). kernel() sums the 8 partial
stat vectors and finalizes out = t/s on the host (the "unshard" step).

Per-core pipeline (~95 us DMA roofline at the measured ~400 GB/s):
  * host pre-packs the shard transposed/tiled so every DMA is a contiguous
    [128, 4096] load; the f32->bf16 cast rides the SWDGE DMA at line rate.
  * per 512-instance tile: one 6-wide PE accumulation group (8 chunk matmuls
    over D=1024, bf16) -> psY[6, 512] in PSUM. Tiles are processed in
    groups of 4 so PE bursts are long enough to hold the HAM clock warm.
  * engine ops need partition bases == 0 mod 32, so the two score rows are
    tanh-ed in place ([6, NT] ACT op) and then each broadcast to a base-0
    tile with a DVE stream_shuffle; e is shuffle-broadcast to 4 partitions
    for the numerator multiply.
  * ACT's accum_out fuses the denominator reduction into the exp; a DVE
    scalar_tensor_tensor with accum_out fuses multiply+reduce for the
    numerator. Per-bag partial columns fold as soon as a bag completes.
"""

import numpy as np

B, N, D, L = 4, 16384, 1024, 4
NCORES = 8
NSH = N // NCORES            # 2048 instances per bag per core
J = B * NSH                  # 8192 flattened rows per core
NT = 512                     # instances per compute tile (1 PSUM bank)
T = J // NT                  # 16 tiles
C8 = D // 128                # 8 contraction chunks of 128
TPB = NSH // NT              # tiles per bag = 4

_NC_CACHE = {}


def _build_nc():
    from concourse import bacc, mybir, tile

    dt = mybir.dt
    act = mybir.ActivationFunctionType
    alu = mybir.AluOpType
    f32 = dt.float32
    bf16 = dt.bfloat16

    nc = bacc.Bacc(
        "TRN2", target_bir_lowering=False, debug=False, num_devices=NCORES
    )

    xsp = nc.dram_tensor("xsp", [T, 128, C8 * NT], bf16, kind="ExternalInput").ap()
    # [128, 48]: per chunk c, cols c*6..c*6+5 = [wf0..wf3, w1, w2]
    wsb = nc.dram_tensor("wsb", [128, C8 * 6], bf16, kind="ExternalInput").ap()
    # per-core partial stats: [s_b (4) | t_{b,l} l-major (16)]
    outp = nc.dram_tensor("out", [1, 5 * B], f32, kind="ExternalOutput").ap()

    with tile.TileContext(nc) as tc:
        with (
            tc.tile_pool(name="const", bufs=1) as constp,
            tc.tile_pool(name="xs", bufs=2) as xpool,
            tc.tile_pool(name="psY", bufs=2, space="PSUM") as ypool,
            tc.tile_pool(name="sm", bufs=3) as smp,
        ):
            w_sb = constp.tile([128, C8 * 6], bf16, tag="w")
            nc.sync.dma_start(w_sb[:], wsb)
            sPP = constp.tile([1, T], f32, tag="sPP")
            sYY = constp.tile([4, T], f32, tag="sYY")
            redP = constp.tile([1, B], f32, tag="redP")
            redY = constp.tile([4, B], f32, tag="redY")

            # group sizes: big bursts keep the PE's HAM clock warm during
            # the stream; the taper at the end stops the last tiles' post-
            # processing chains from bunching up after the final matmuls
            GROUPS = [4, 4, 4, 2, 1, 1]
            assert sum(GROUPS) == T
            starts = [sum(GROUPS[:i]) for i in range(len(GROUPS))]
            for gi, g0 in enumerate(starts):
              grp = range(g0, g0 + GROUPS[gi])
              xts, psYs = [], []
              for t in grp:
                xt = xpool.tile([128, C8 * NT], bf16, tag=f"xt{t % 4}")
                nc.gpsimd.dma_start(xt[:], xsp[t])  # f32 -> bf16 cast DMA
                xts.append(xt)
              for t in grp:
                xt = xts[t - g0]
                psY = ypool.tile([6, NT], f32, tag=f"y{t % 4}")
                psYs.append(psY)
                for c in range(C8):
                    nc.tensor.matmul(
                        psY[:],
                        w_sb[:, c * 6 : (c + 1) * 6],
                        xt[:, c * NT : (c + 1) * NT],
                        start=(c == 0),
                        stop=(c == C8 - 1),
                    )
              for t in grp:
                bg = t // TPB  # bag index for this tile
                psY = psYs[t - g0]

                # tanh(y/2) of all 6 rows (only rows 4,5 used; host scaled
                # w1 by 2 so row4 gives tanh(s1) under the uniform /2 scale)
                tts = smp.tile([32, NT], bf16, tag="tts")
                nc.scalar.activation(tts[0:6, :], psY[:], act.Tanh, scale=0.5)
                # shuffle-broadcast rows 4 and 5 each to a base-0 tile
                uA = smp.tile([32, NT], bf16, tag="uA")
                nc.vector.stream_shuffle(uA[:], tts[:], [4] * 32)
                uB = smp.tile([32, NT], bf16, tag="uB")
                nc.vector.stream_shuffle(uB[:], tts[:], [5] * 32)
                # v = tanh(s1) * (tanh(s2/2) + 1) == 2*a ; e = exp(v/2)
                # invalid instances have zeroed xs -> e = exp(0) = 1 exactly;
                # the host subtracts the per-bag invalid count from sum(e)
                v = smp.tile([1, NT], f32, tag="v")
                nc.vector.scalar_tensor_tensor(
                    v[:], uB[0:1, :], 1.0, uA[0:1, :], alu.add, alu.mult
                )
                # e = exp(v/2); denominator partial = sum_n e (accum_out)
                e = smp.tile([32, NT], bf16, tag="e")
                nc.scalar.activation(
                    e[0:1, :], v[:], act.Exp, scale=0.5,
                    accum_out=sPP[0:1, t : t + 1],
                )
                # broadcast e to 4 partitions
                ebs = smp.tile([32, NT], bf16, tag="ebs")
                nc.vector.stream_shuffle(ebs[:], e[:], [0] * 32)

                # numerator partials: sYY[:, t] = sum_n psY[0:4]*e
                jY = smp.tile([4, NT], f32, tag="jY")
                nc.vector.scalar_tensor_tensor(
                    jY[:], psY[0:4, :], 1.0, ebs[0:4, :], alu.mult, alu.mult,
                    accum_out=sYY[0:4, t : t + 1],
                )
                if t % TPB == TPB - 1:
                    # bag bg complete: fold its 4 partial columns immediately
                    nc.vector.tensor_reduce(
                        redP[0:1, bg : bg + 1],
                        sPP[0:1, bg * TPB : (bg + 1) * TPB],
                        mybir.AxisListType.X,
                        alu.add,
                    )
                    nc.vector.tensor_reduce(
                        redY[0:4, bg : bg + 1],
                        sYY[0:4, bg * TPB : (bg + 1) * TPB],
                        mybir.AxisListType.X,
                        alu.add,
                    )
                    # ship this bag's output columns now, off the tail path
                    # (out layout: flat[0:4]=s_b; flat[4+l*4+b]=t_{b,l})
                    nc.scalar.dma_start(
                        outp[0:1, bg : bg + 1], redP[0:1, bg : bg + 1]
                    )
                    nc.sync.dma_start(
                        outp[0:1, B : 5 * B].rearrange("p (l b) -> l (p b)", l=4)[
                            :, bg : bg + 1
                        ],
                        redY[0:4, bg : bg + 1],
                    )



    nc.compile()
    return nc


def _get_nc():
    if "nc" not in _NC_CACHE:
        _NC_CACHE["nc"] = _build_nc()
    return _NC_CACHE["nc"]


def _make_in_maps(xs, valid, w1, w2, wf):
    import ml_dtypes

    validf = valid.astype(np.float32)
    xsz = (xs.astype(np.float32) * validf[..., None]).astype(np.float32)
    # [D, 6] = [wf.T | w1 | w2], packed per 128-chunk: (p, c*6+j) = W6[c*128+p, j]
    W6 = np.concatenate(
        [wf.astype(np.float32).T, 2.0 * w1.astype(np.float32), w2.astype(np.float32)],
        axis=1,
    )
    wsb = np.ascontiguousarray(
        W6.reshape(C8, 128, 6).transpose(1, 0, 2).reshape(128, C8 * 6)
    ).astype(ml_dtypes.bfloat16)


    in_maps = []
    for c in range(NCORES):
        sh = xsz[:, c * NSH : (c + 1) * NSH, :].reshape(J, D)
        xt = sh.T  # [D, J]
        packed = (
            np.ascontiguousarray(xt.reshape(C8, 128, T, NT).transpose(2, 1, 0, 3))
            .reshape(T, 128, C8 * NT)
            .astype(ml_dtypes.bfloat16)
        )
        in_maps.append({"xsp": packed, "wsb": wsb})
    return in_maps


def _run(xs, valid, w1, w2, wf, trace=False, **kwargs):
    from concourse import bass_utils

    nc = _get_nc()
    in_maps = _make_in_maps(xs, valid, w1, w2, wf)
    res = bass_utils.run_bass_kernel_spmd(
        nc, in_maps, core_ids=list(range(NCORES)), trace=trace, **kwargs
    )
    return res


def _combine(res, valid):
    """Sum per-core partial stats (flash-style unshard) and finalize t/s.

    Invalid instances were zeroed on the device input, so each contributes
    exp(0) = 1 to the denominator partials; subtract their count here.
    """
    tot = np.zeros(5 * B, np.float64)
    for c in range(NCORES):
        tot += np.asarray(res.results[c]["out"]).reshape(5 * B).astype(np.float64)
    n_invalid = (~valid.astype(bool)).sum(axis=1).astype(np.float64)  # [b]
    s = tot[0:B] - n_invalid                      # [b]
    t = tot[B:].reshape(L, B).T                   # [b, l]
    return (t / s[:, None]).astype(np.float32)


def kernel(xs, valid, w1, w2, wf):
    xs, valid, w1, w2, wf = (np.asarray(a) for a in (xs, valid, w1, w2, wf))
    res = _run(xs, valid, w1, w2, wf, trace=False)
    return _combine(res, valid)



# revision 4
# speedup vs baseline: 2.3479x; 1.7013x over previous
"""ABMIL attention pooling on 8 TRN2 NeuronCores (Bass/Tile, SPMD).

Reference (per bag b over N=16384 instances):
    a_n   = tanh(x_n . w1) * sigmoid(x_n . w2)     gated attention score
    att   = softmax over valid n of a              (invalid -> -1e4)
    out_b = sum_n att_n * (x_n @ wf.T)             weighted pooling + proj

Folds that make this memory-bound (read xs exactly once, in fp8):
  * out = (sum_n att_n x_n) @ wf.T == sum_n att_n (x_n @ wf.T): the only
    large compute is ONE matmul y = xs @ [wf.T | 2*w1 | w2] ([N, 6]).
  * scores lie in (-1, 1), so softmax needs no max-subtraction:
    out = sum(e*y) / sum_valid(e) with e = exp(a).
  * sigmoid(x) = 0.5*(1 + tanh(x/2)); w1 is pre-scaled by 2 on the host so
    one ACT scale (0.5) serves both tanh rows.
  * invalid instances are zeroed in the packed xs, so y rows are 0 and
    e = exp(0) = 1 exactly; the host subtracts the per-bag invalid count
    from the denominator. No mask tensor reaches the device.
  * xs is staged as fp8 E3M4 (host cast): 8 MiB per core instead of 32.
    Host-sim rel_err vs f64 reference: 6.9e-3 (bf16 weights), well under
    the 2e-2 gate. E4M3 (2.3e-2) fails; E3M4's 4 mantissa bits and [2^-6,
    15.5] range fit N(0,1) data. Weights stay bf16 (tiny, exact-ish).

Matmul orientation (the key restructure vs the 104us/74us baselines):
  the 128x128 x-block is the STATIONARY operand and the [128, 6] weight
  chunk is the MOVING operand, so psY = x_blk.T @ W lands TRANSPOSED:
  [128 instances, 6] per block. Consequences:
  * LDWEIGHTS (x-block load) rides fast-weight-load (4 fp8 cols/cycle) and
    overlaps the previous matmul (ping-pong weight planes), so the PE pipe
    runs ~32+6 cycles per block-chunk instead of streaming 512 moving
    columns -- and the PE p-state ramp stops mattering.
  * all softmax/pooling post-work runs at [128, 16] shapes (instances on
    partitions): ~16 lane-cycles per op instead of 512. DVE drops from
    ~38us (shuffle-heavy [*, 512] ops) to ~3us, ACT from ~25us to ~2us.

Sharding (flash-attention style): instance dim N split 8 ways; each core
streams its shard once and emits 20 floats (per bag: sum e, sum e*y).
kernel() sums the partials and finalizes t/s on the host.
"""

import numpy as np

B, N, D, L = 4, 16384, 1024, 4
NCORES = 8
NSH = N // NCORES            # 2048 instances per bag per core
J = B * NSH                  # 8192 flattened rows per core
C8 = D // 128                # 8 contraction chunks of 128
NBLK = J // 128              # 64 n-blocks of 128 instances
BPB = NSH // 128             # 16 blocks per bag
TD = 16                      # DMA tiles (4 n-blocks each)
BPT = NBLK // TD             # 4 blocks per DMA tile
LOOK = 6                     # x tiles in flight

_NC_CACHE = {}


def _build_nc():
    from concourse import bacc, mybir, tile

    dt = mybir.dt
    act = mybir.ActivationFunctionType
    alu = mybir.AluOpType
    f32 = dt.float32
    bf16 = dt.bfloat16
    f8 = dt.float8e3

    nc = bacc.Bacc(
        "TRN2", target_bir_lowering=False, debug=False, num_devices=NCORES
    )

    # [tile, d-partition, (4 blocks x 8 chunks x 128 instances)] fp8 e3m4
    xsp = nc.dram_tensor("xsp", [TD, 128, BPT * C8 * 128], f8, kind="ExternalInput").ap()
    # [128, 48]: per chunk c, cols c*6..c*6+5 = [wf0..wf3, 2*w1, w2]
    wsb = nc.dram_tensor("wsb", [128, C8 * 6], bf16, kind="ExternalInput").ap()
    # per-core partials: [0:4] = sum e per bag; [4 + b*4 + l] = sum e*y
    outp = nc.dram_tensor("out", [1, 5 * B], f32, kind="ExternalOutput").ap()

    with tile.TileContext(nc) as tc:
        with (
            tc.tile_pool(name="const", bufs=1) as constp,
            tc.tile_pool(name="xs", bufs=1) as xpool,
            tc.tile_pool(name="psY", bufs=1, space="PSUM") as ypool,
            tc.tile_pool(name="sm", bufs=2) as smp,
        ):
            w_sb = constp.tile([128, C8 * 6], bf16, tag="w")
            nc.sync.dma_start(w_sb[:], wsb)
            ones = constp.tile([128, 1], f32, tag="ones")
            nc.vector.memset(ones[:], 1.0)
            # accum columns: [0:4] denom per bag, [4 + b*4 + l] numerators
            sAcc = constp.tile([128, 5 * B], f32, tag="sAcc")

            psY = [
                ypool.tile([128, 512], f32, tag=f"bag{b}", name=f"psY{b}")
                for b in range(B)
            ]

            for t in range(TD):
                xt = xpool.tile(
                    [128, BPT * C8 * 128], f8, tag=f"x{t % LOOK}", name=f"xt{t}"
                )
                nc.gpsimd.dma_start(xt[:], xsp[t])
                for bb in range(BPT):
                    blk = t * BPT + bb
                    bg, j = blk // BPB, blk % BPB
                    for c in range(C8):
                        nc.tensor.matmul(
                            psY[bg][:, j * 6 : (j + 1) * 6],
                            xt[:, (bb * C8 + c) * 128 : (bb * C8 + c + 1) * 128],
                            w_sb[:, c * 6 : (c + 1) * 6],
                            start=(c == 0),
                            stop=(c == C8 - 1),
                        )
                if (t * BPT + BPT) % BPB == 0:
                    # bag bg's 16 blocks done: post-process at [128, 16] shapes
                    py = psY[bg][:, 0 : BPB * 6].rearrange("p (g k) -> p g k", k=6)
                    # tanh of both score cols (s1 at k=4 scaled 2x on host)
                    tts = smp.tile([128, BPB, 2], bf16, tag="tts")
                    nc.scalar.activation(tts[:], py[:, :, 4:6], act.Tanh, scale=0.5)
                    # v = tanh(x.w1) * (tanh(x.w2 / 2) + 1) == 2a
                    v = smp.tile([128, BPB], f32, tag="v")
                    nc.vector.scalar_tensor_tensor(
                        v[:], tts[:, :, 1], 1.0, tts[:, :, 0], alu.add, alu.mult
                    )
                    # e = exp(v/2); accumulate denominator partial for bag
                    e_b = smp.tile([128, BPB], bf16, tag="e")
                    nc.scalar.activation(
                        e_b[:], v[:], act.Exp, scale=0.5,
                        accum_out=sAcc[:, bg : bg + 1],
                    )
                    # numerators: sum_n e_n * y_nl
                    for l in range(L):
                        jnk = smp.tile([128, BPB], bf16, tag=f"jnk{l}")
                        nc.vector.scalar_tensor_tensor(
                            jnk[:], py[:, :, l], 1.0, e_b[:], alu.mult, alu.mult,
                            accum_out=sAcc[:, B + bg * L + l : B + bg * L + l + 1],
                        )

            # fold partitions: [1, 20] = ones.T @ sAcc, then ship out
            psOut = ypool.tile([1, 5 * B], f32, tag="out")
            nc.tensor.matmul(psOut[:], ones[:], sAcc[:], start=True, stop=True)
            outSb = constp.tile([1, 5 * B], f32, tag="outSb")
            nc.scalar.copy(outSb[:], psOut[:])
            nc.sync.dma_start(outp, outSb[:])

    nc.compile()
    return nc


def _get_nc():
    if "nc" not in _NC_CACHE:
        _NC_CACHE["nc"] = _build_nc()
    return _NC_CACHE["nc"]


def _make_in_maps(xs, valid, w1, w2, wf):
    import ml_dtypes

    validf = valid.astype(np.float32)
    xsz = xs.astype(np.float32) * validf[..., None]
    # [D, 6] = [wf.T | 2*w1 | w2], packed per 128-chunk: (p, c*6+j) = W6[c*128+p, j]
    W6 = np.concatenate(
        [wf.astype(np.float32).T, 2.0 * w1.astype(np.float32), w2.astype(np.float32)],
        axis=1,
    )
    wsb = np.ascontiguousarray(
        W6.reshape(C8, 128, 6).transpose(1, 0, 2).reshape(128, C8 * 6)
    ).astype(ml_dtypes.bfloat16)

    in_maps = []
    for c in range(NCORES):
        sh = xsz[:, c * NSH : (c + 1) * NSH, :].reshape(J, D)
        # [blk, nn, chunk, dd] -> [tile, dd, (blk-in-tile, chunk, nn)]
        a = sh.reshape(NBLK, 128, C8, 128).transpose(0, 2, 3, 1)
        a = (
            a.reshape(TD, BPT, C8, 128, 128)
            .transpose(0, 3, 1, 2, 4)
            .reshape(TD, 128, BPT * C8 * 128)
        )
        packed = np.ascontiguousarray(a).astype(ml_dtypes.float8_e3m4)
        in_maps.append({"xsp": packed, "wsb": wsb})
    return in_maps


def _run(xs, valid, w1, w2, wf, trace=False, **kwargs):
    from concourse import bass_utils

    nc = _get_nc()
    in_maps = _make_in_maps(xs, valid, w1, w2, wf)
    res = bass_utils.run_bass_kernel_spmd(
        nc, in_maps, core_ids=list(range(NCORES)), trace=trace, **kwargs
    )
    return res


def _combine(res, valid):
    """Sum per-core partial stats (flash-style unshard) and finalize t/s.

    Invalid instances were zeroed on the device input, so each contributes
    exp(0) = 1 to the denominator partials; subtract their count here.
    """
    tot = np.zeros(5 * B, np.float64)
    for c in range(NCORES):
        tot += np.asarray(res.results[c]["out"]).reshape(5 * B).astype(np.float64)
    n_invalid = (~valid.astype(bool)).sum(axis=1).astype(np.float64)  # [b]
    s = tot[0:B] - n_invalid                      # [b]
    t = tot[B:].reshape(B, L)                     # [b, l]
    return (t / s[:, None]).astype(np.float32)


def kernel(xs, valid, w1, w2, wf):
    xs, valid, w1, w2, wf = (np.asarray(a) for a in (xs, valid, w1, w2, wf))
    res = _run(xs, valid, w1, w2, wf, trace=False)
    return _combine(res, valid)
